# revision 1
# baseline (speedup 1.0000x reference)
"""Trainium2 Bass kernel for 16-head causal MHA (B=4, S=2048, E=1024, D=64).

Sharding: 8 cores = 4 batches x 2 head-halves. Each core computes QKV
projections + causal attention for 8 heads of one batch plus the partial
output projection for its head-half's columns of Wo. Host sums the two
partials per batch and adds the effective bias (bo + bv-through-Wo, since
softmax rows sum to 1 the V-bias contribution is a constant vector).

All matmuls run in fp32r (full PE rate, ~1e-4 relative rounding).
V is augmented with a ones column so the ctx matmul's extra output row
accumulates the softmax denominator exactly in PSUM.
"""
import numpy as np

B, S, E = 4, 2048, 1024
H, D = 16, 64
NP = 4     # head-pairs per core (2 heads packed in the transposed projections)
KT = 8     # E / 128 contraction tiles
NQB = 4    # q blocks of 512
NTT = 16   # t tiles of 128

_NC = None


def _build():
    import concourse.bacc as bacc
    import concourse.tile as tile
    from concourse import mybir
    from concourse.masks import make_identity

    f32, f32r = mybir.dt.float32, mybir.dt.float32r
    Act = mybir.ActivationFunctionType

    nc = bacc.Bacc("TRN2")
    X = nc.dram_tensor("x", [S, E], f32, kind="ExternalInput")
    WQ = nc.dram_tensor("wq", [NP, KT, 128, 128], f32, kind="ExternalInput")
    WK = nc.dram_tensor("wk", [NP, KT, 128, 128], f32, kind="ExternalInput")
    WV = nc.dram_tensor("wv", [NP, KT, 128, 128], f32, kind="ExternalInput")
    BQ = nc.dram_tensor("bq", [NP, 128, 1], f32, kind="ExternalInput")
    BK = nc.dram_tensor("bk", [NP, 128, 1], f32, kind="ExternalInput")
    WO = nc.dram_tensor("wo", [NP, 128, E], f32, kind="ExternalInput")
    TRI = nc.dram_tensor("tri", [128, 128], f32, kind="ExternalInput")
    OUT = nc.dram_tensor("out", [S, E], f32, kind="ExternalOutput")

    with tile.TileContext(nc) as tc:
        with tc.tile_pool(name="persist", bufs=1) as pers:
            ident_f = pers.tile([128, 128], f32)
            make_identity(nc, ident_f)
            ident_r = pers.tile([128, 128], f32r)
            nc.vector.tensor_copy(ident_r, ident_f)
            ones_f = pers.tile([128, 1], f32)
            nc.vector.memset(ones_f, 1.0)
            ones16 = pers.tile([128, NTT, 1], f32)
            nc.vector.memset(ones16, 1.0)
            onesrow_f = pers.tile([1, 64], f32)
            nc.vector.memset(onesrow_f, 1.0)
            ones_row = pers.tile([1, 64], f32r)
            nc.vector.tensor_copy(ones_row, onesrow_f)
            tri_f = pers.tile([128, 128], f32)
            nc.sync.dma_start(tri_f, TRI.ap())
            tri_r = pers.tile([128, 128], f32r)
            nc.vector.tensor_copy(tri_r, tri_f)
            zeros_f = pers.tile([128, 384], f32)
            nc.vector.memset(zeros_f, 0.0)
            zeros_r = pers.tile([128, 384], f32r)
            nc.vector.tensor_copy(zeros_r, zeros_f)

            bq_t, bk_t = [], []
            for p in range(NP):
                t1 = pers.tile([128, 1], f32, name=f"bq_t{p}")
                nc.sync.dma_start(t1, BQ.ap()[p])
                bq_t.append(t1)
                t2 = pers.tile([128, 1], f32, name=f"bk_t{p}")
                nc.sync.dma_start(t2, BK.ap()[p])
                bk_t.append(t2)

            with tc.tile_pool(name="xtp", bufs=1) as xtp, \
                 tc.tile_pool(name="ctxp", bufs=1) as ctxp:
                xT = [xtp.tile([128, S], f32r, name=f"xT{i}") for i in range(KT)]
                ctxN = [ctxp.tile([128, S], f32r, name=f"ctxN{i}") for i in range(NP)]

                # ---- Phase A: x -> x^T (fp32r) via PE transpose ----
                # column-major load: xT[k] completes after one 1MB DMA, so
                # the k-ordered QKV matmuls can start almost immediately
                xcols = X.ap().rearrange("(st p) e -> p st e", p=128)
                with tc.tile_pool(name="stA", bufs=2) as sa, \
                     tc.tile_pool(name="psA", bufs=4, space="PSUM") as pA:
                    for k in range(KT):
                        colblk = sa.tile([128, NTT, 128], f32)
                        if k == 0:
                            for q4 in range(4):
                                nc.sync.dma_start(
                                    colblk[:, q4 * 4:(q4 + 1) * 4, :],
                                    xcols[:, q4 * 4:(q4 + 1) * 4, k * 128:(k + 1) * 128])
                        else:
                            nc.sync.dma_start(colblk, xcols[:, :, k * 128:(k + 1) * 128])
                        for st in range(NTT):
                            tp = pA.tile([128, 128], f32)
                            nc.tensor.transpose(tp, colblk[:, st, :], ident_f)
                            nc.vector.tensor_copy(xT[k][:, st * 128:(st + 1) * 128], tp)

                # ---- Phases B+C merged: per pair, QKV projection then attention ----
                with tc.tile_pool(name="qtp", bufs=2) as qtp, \
                     tc.tile_pool(name="ktp", bufs=2) as ktp, \
                     tc.tile_pool(name="vnp", bufs=2) as vnp, \
                     tc.tile_pool(name="stB", bufs=3) as sb_, \
                     tc.tile_pool(name="vt2", bufs=1) as vt2p, \
                     tc.tile_pool(name="expp", bufs=6) as expp, \
                     tc.tile_pool(name="rp", bufs=4) as rp, \
                     tc.tile_pool(name="psB", bufs=4, space="PSUM") as pB, \
                     tc.tile_pool(name="psCTX", bufs=1, space="PSUM") as psCTX:
                    for p in range(NP):
                        qt = qtp.tile([128, S], f32r, name="qt")
                        kt = ktp.tile([128, S], f32r, name="kt")
                        vn = vnp.tile([128, 2, NTT, 65], f32r, name="vn")
                        vt2 = vt2p.tile([128, S], f32r)

                        # QKV projections (transposed, 2-head packed)
                        for W_, bias_, dest in (
                            (WQ, bq_t[p], qt),
                            (WK, bk_t[p], kt),
                            (WV, None, vt2),
                        ):
                            wrs = []
                            for k in range(KT):
                                wf = sb_.tile([128, 128], f32, name="wf", bufs=3)
                                nc.sync.dma_start(wf, W_.ap()[p, k])
                                wr = sb_.tile([128, 128], f32r, name="wr", bufs=10)
                                nc.vector.tensor_copy(wr, wf)
                                wrs.append(wr)
                            for half in range(2):
                                pss = [pB.tile([128, 512], f32, name="pss", bufs=2)
                                       for _ in range(2)]
                                for k in range(KT):
                                    for i in range(2):
                                        nb = 2 * half + i
                                        nc.tensor.matmul(
                                            pss[i], wrs[k],
                                            xT[k][:, nb * 512:(nb + 1) * 512],
                                            start=(k == 0), stop=(k == KT - 1),
                                        )
                                for i in range(2):
                                    nb = 2 * half + i
                                    dslc = dest[:, nb * 512:(nb + 1) * 512]
                                    if bias_ is not None:
                                        nc.vector.tensor_scalar_add(dslc, pss[i], bias_)
                                    else:
                                        nc.vector.tensor_copy(dslc, pss[i])
                        # V back to natural [t, d] layout, split per head + ones col
                        for h in range(2):
                            nc.vector.tensor_copy(vn[:, h, :, 64:65], ones16)
                        for tt in range(NTT):
                            tp2 = pB.tile([128, 128], f32r, name="sc", bufs=4)
                            nc.tensor.transpose(tp2, vt2[:, tt * 128:(tt + 1) * 128], ident_r)
                            for h in range(2):
                                nc.vector.tensor_copy(
                                    vn[:, h, tt, 0:64], tp2[:, h * 64:(h + 1) * 64])

                        # attention for this pair
                        for qb in range(NQB):
                            T = 4 * (qb + 1)  # causal: t-tiles 0..T-1
                            cps = [psCTX.tile([65, 512], f32, name=f"cps{h}")
                                   for h in range(2)]
                            prev_exp = None
                            for tt in range(T):
                                scs = []
                                for h in range(2):
                                    sc = pB.tile([128, 512], f32, name="sc", bufs=4)
                                    nc.tensor.matmul(
                                        sc,
                                        kt[h * 64:(h + 1) * 64, tt * 128:(tt + 1) * 128],
                                        qt[h * 64:(h + 1) * 64, qb * 512:(qb + 1) * 512],
                                        start=True, stop=True,
                                    )
                                    scs.append(sc)
                                if prev_exp is not None:
                                    for h in range(2):
                                        nc.tensor.matmul(
                                            cps[h], vn[:, h, tt - 1, :], prev_exp[h],
                                            start=(tt - 1 == 0), stop=False,
                                        )
                                j = tt - 4 * qb  # >=0 on diagonal tiles
                                cur = []
                                for h in range(2):
                                    ex = expp.tile([128, 512], f32r)
                                    if j >= 1:
                                        nc.gpsimd.tensor_copy(
                                            ex[:, 0:j * 128], zeros_r[:, 0:j * 128])
                                    if j >= 0:
                                        nc.scalar.activation(
                                            ex[:, j * 128:512], scs[h][:, j * 128:512],
                                            Act.Exp, scale=0.125)
                                        nc.vector.tensor_mul(
                                            ex[:, j * 128:(j + 1) * 128],
                                            ex[:, j * 128:(j + 1) * 128], tri_r)
                                    else:
                                        nc.scalar.activation(ex, scs[h], Act.Exp, scale=0.125)
                                    cur.append(ex)
                                prev_exp = cur
                            for h in range(2):
                                nc.tensor.matmul(
                                    cps[h], vn[:, h, T - 1, :], prev_exp[h],
                                    start=(T - 1 == 0), stop=True,
                                )
                            # evict cps to SBUF fast (frees PSUM banks), then
                            # denominators (row 64) -> bcast -> reciprocal -> normalize
                            for h in range(2):
                                csb = rp.tile([65, 512], f32, name="csb", bufs=3)
                                nc.scalar.copy(csb, cps[h])
                                rh = rp.tile([1, 512], f32r, name="rh")
                                nc.vector.tensor_copy(rh, csb[64:65, :])
                                rb = pB.tile([64, 512], f32, name="sc", bufs=4)
                                nc.tensor.matmul(rb, ones_row, rh, start=True, stop=True)
                                rbs = rp.tile([64, 512], f32, name="rbs")
                                nc.vector.reciprocal(rbs, rb)
                                nc.vector.tensor_mul(
                                    ctxN[p][h * 64:(h + 1) * 64, qb * 512:(qb + 1) * 512],
                                    csb[0:64, :], rbs,
                                )

                # ---- Phase D: output projection (partial, this head-half) ----
                with tc.tile_pool(name="stD", bufs=3) as sd, \
                     tc.tile_pool(name="wo2", bufs=1) as wop, \
                     tc.tile_pool(name="psD", bufs=4, space="PSUM") as pD:
                    wo_r = []
                    for p in range(NP):
                        wf2 = sd.tile([128, E], f32, name="wf2")
                        nc.sync.dma_start(wf2, WO.ap()[p])
                        wr2 = wop.tile([128, E], f32r, name=f"wo2_{p}")
                        nc.vector.tensor_copy(wr2, wf2)
                        wo_r.append(wr2)
                    for qt_i in range(NTT):
                        ob = sd.tile([128, E], f32, name="ob")
                        for eh in range(2):
                            ps = pD.tile([128, 512], f32, name="psd")
                            for p in range(NP):
                                nc.tensor.matmul(
                                    ps,
                                    ctxN[p][:, qt_i * 128:(qt_i + 1) * 128],
                                    wo_r[p][:, eh * 512:(eh + 1) * 512],
                                    start=(p == 0), stop=(p == NP - 1),
                                )
                            nc.vector.tensor_copy(ob[:, eh * 512:(eh + 1) * 512], ps)
                        nc.sync.dma_start(OUT.ap()[qt_i * 128:(qt_i + 1) * 128, :], ob)

    nc.finalize()
    return nc


def _get_nc():
    global _NC
    if _NC is None:
        _NC = _build()
    return _NC


def _pack_w(Wh):
    # [8, E, D] -> [NP, KT, 128, 128]; out[p,k,i,j] = Wh[2p + j//64, k*128+i, j%64]
    w = Wh.reshape(NP, 2, E, D)
    w = np.transpose(w, (0, 2, 1, 3)).reshape(NP, E, 128)
    w = w.reshape(NP, KT, 128, 128)
    return np.ascontiguousarray(w, dtype=np.float32)


def kernel(x, Wq, bq, Wk, bk, Wv, bv, Wo, bo):
    from concourse.bass_utils import run_bass_kernel_spmd

    x = np.asarray(x, dtype=np.float32)
    Wq = np.asarray(Wq, dtype=np.float32)
    bq = np.asarray(bq, dtype=np.float32)
    Wk = np.asarray(Wk, dtype=np.float32)
    bk = np.asarray(bk, dtype=np.float32)
    Wv = np.asarray(Wv, dtype=np.float32)
    bv = np.asarray(bv, dtype=np.float32)
    Wo = np.asarray(Wo, dtype=np.float32)
    bo = np.asarray(bo, dtype=np.float32)

    nc = _get_nc()

    tri = (np.arange(128)[None, :] >= np.arange(128)[:, None]).astype(np.float32)
    tri = np.ascontiguousarray(tri)

    half_maps = []
    for hh in range(2):
        hsel = slice(hh * 8, hh * 8 + 8)
        half_maps.append({
            "wq": _pack_w(Wq[hsel]),
            "wk": _pack_w(Wk[hsel]),
            "wv": _pack_w(Wv[hsel]),
            "bq": np.ascontiguousarray(bq[hsel].reshape(NP, 128, 1)),
            "bk": np.ascontiguousarray(bk[hsel].reshape(NP, 128, 1)),
            "wo": np.ascontiguousarray(
                Wo[:, hh * 512:(hh + 1) * 512].T.reshape(NP, 128, E),
                dtype=np.float32),
            "tri": tri,
        })
    in_maps = []
    for c in range(8):
        b, hh = divmod(c, 2)
        in_maps.append({"x": np.ascontiguousarray(x[b]), **half_maps[hh]})

    res = run_bass_kernel_spmd(nc, in_maps, core_ids=list(range(8)))
    parts = np.stack([res.results[c]["out"] for c in range(8)])  # [8, S, E]

    # effective bias: bo plus bv routed through Wo (softmax rows sum to 1)
    bo_eff = bo + bv.reshape(-1) @ Wo.T
    out = parts.reshape(B, 2, S, E).sum(axis=1) + bo_eff[None, None, :]
    return out.astype(np.float32)



# revision 14
# speedup vs baseline: 4.0095x; 4.0095x over previous
"""Trainium2 Bass kernel for 16-head causal MHA (B=4, S=2048, E=1024, D=64).

Sharding: 8 cores = 4 batches x 2 head-halves; core c handles batch c//2,
head-half c%2. All wire traffic is bf16 and deduplicated with on-device
collectives so each unique byte crosses the host<->device link once:

  - x: core c ships 1/8 of x (1024 rows); a pair AllGather {2b, 2b+1}
    reconstructs the full x[b] on device.
  - Wq/Wk/Wv: core c ships one head-pair slab; an AllGather over
    {0,2,4,6} / {1,3,5,7} gives each core its half's 4 pairs, at local
    pair indices (the program is identical on every core; all per-core
    differences are in the input data).
  - Wo: transposed + column-halved per head-half group, sharded 2 pairs
    per core, 8-way AllGather -> global-pair-indexed [8,128,512] tiles.
  - ctx: after each pair's attention, a pair AllGather shares its ctx so
    each core computes its own 512 output columns of out[b] (the output
    projection needs all 16 heads).

Each core returns out[b][:, hh*512:(hh+1)*512] in bf16 (pre-bias); the
host interleaves column halves and adds bo_eff = bo + bv @ Wo.T (softmax
rows sum to 1, so the V-bias routes through Wo as a constant vector).

Matmul inputs are bf16 (full PE rate), accumulation in f32 PSUM.
"""
import numpy as np

B, S, E = 4, 2048, 1024
H, D = 16, 64
NP = 4     # head-pairs per core (2 heads packed per 128-wide tile)
KT = 8     # E / 128 contraction tiles
NQB = 4    # q blocks of 512
NTT = 16   # t tiles of 128

_NC = None
_RUNNER = None
_SIM_NO_CC = False  # probe: replace collectives with local DMAs (timing only)


def _build():
    import concourse.bacc as bacc
    import concourse.tile as tile
    from concourse import mybir
    from concourse.masks import make_identity, make_upper_triangular

    f32, f32r = mybir.dt.float32, mybir.dt.float32r
    bf16 = mybir.dt.bfloat16
    Act = mybir.ActivationFunctionType

    nc = bacc.Bacc("TRN2", num_devices=8)
    XS = nc.dram_tensor("xs", [1024, E], bf16, kind="ExternalInput")
    WQKVS = nc.dram_tensor("wqkvs", [3, KT, 128, 128], bf16, kind="ExternalInput")
    WOS = nc.dram_tensor("wos", [2, 128, 512], bf16, kind="ExternalInput")
    BQK = nc.dram_tensor("bqk", [2, NP, 128, 1], f32, kind="ExternalInput")
    OUT = nc.dram_tensor("out", [S, 512], bf16, kind="ExternalOutput")

    PAIR_GROUPS = [[0, 1], [2, 3], [4, 5], [6, 7]]
    HALF_GROUPS = [[0, 2, 4, 6], [1, 3, 5, 7]]

    def allgather(groups, in_tile, out_tile):
        if _SIM_NO_CC:
            n = len(groups[0])
            chunk = out_tile.shape[0] // n
            for r in range(n):
                dst = (out_tile[r] if chunk == 1
                       else out_tile[r * chunk:(r + 1) * chunk])
                nc.sync.dma_start(dst, in_tile)
            return
        nc.gpsimd.collective_compute(
            "AllGather", mybir.AluOpType.bypass,
            replica_groups=groups, ins=[in_tile.opt()], outs=[out_tile.opt()])

    with tile.TileContext(nc) as tc:
        with tc.tile_pool(name="dram", bufs=1, space="DRAM") as dp, \
             tc.tile_pool(name="persist", bufs=1) as pers:
            # ---- DRAM scratch: collective bounces + gather outputs ----
            xsb = dp.tile([1024, E], bf16, name="xsb")
            xg = dp.tile([2, 1024, E], bf16, name="xg")
            wqkvb = dp.tile([3, KT, 128, 128], bf16, name="wqkvb")
            wqkvg = dp.tile([NP, 3, KT, 128, 128], bf16, name="wqkvg")
            wob = dp.tile([2, 128, 512], bf16, name="wob")
            wog = dp.tile([8, 128, 512], bf16, name="wog")
            cin = [dp.tile([128, S], bf16, name=f"cin{p}") for p in range(NP)]
            cg = [dp.tile([2, 128, S], bf16, name=f"cg{p}") for p in range(NP)]

            # bounce + input gathers; x first (everything depends on it)
            nc.sync.dma_start(xsb, XS.ap())
            allgather(PAIR_GROUPS, xsb, xg)
            nc.sync.dma_start(wqkvb, WQKVS.ap())
            allgather(HALF_GROUPS, wqkvb, wqkvg)
            nc.sync.dma_start(wob, WOS.ap())
            allgather(HALF_GROUPS, wob, wog)

            # ---- persistent SBUF constants ----
            ident_f = pers.tile([128, 128], f32)
            make_identity(nc, ident_f)
            ident_b = pers.tile([128, 128], bf16)
            nc.vector.tensor_copy(ident_b, ident_f)
            ones16 = pers.tile([128, NTT, 1], bf16)
            nc.vector.memset(ones16, 1.0)
            onesrow_f = pers.tile([1, 64], f32)
            nc.vector.memset(onesrow_f, 1.0)
            ones_row = pers.tile([1, 64], f32r)
            nc.vector.tensor_copy(ones_row, onesrow_f)
            tri_f = pers.tile([128, 128], f32)
            make_upper_triangular(nc, tri_f, val=1.0, diag=True)
            tri_b = pers.tile([128, 128], bf16)
            nc.vector.tensor_copy(tri_b, tri_f)
            zeros_b = pers.tile([128, 384], bf16)
            nc.vector.memset(zeros_b, 0.0)

            bq_t, bk_t = [], []
            for p in range(NP):
                t1 = pers.tile([128, 1], f32, name=f"bq_t{p}")
                nc.sync.dma_start(t1, BQK.ap()[0, p])
                bq_t.append(t1)
                t2 = pers.tile([128, 1], f32, name=f"bk_t{p}")
                nc.sync.dma_start(t2, BQK.ap()[1, p])
                bk_t.append(t2)

            with tc.tile_pool(name="xtp", bufs=1) as xtp:
                xT = [xtp.tile([128, S], bf16, name=f"xT{i}") for i in range(KT)]

                # ---- Phase A: x -> x^T (bf16) via PE transpose ----
                with tc.tile_pool(name="stA", bufs=2) as sa, \
                     tc.tile_pool(name="psA", bufs=4, space="PSUM") as pA:
                    for k in range(KT):
                        colblk = sa.tile([128, NTT, 128], bf16)
                        for h2 in range(2):
                            src = xg[h2].rearrange("(st p) e -> p st e", p=128)
                            nc.sync.dma_start(
                                colblk[:, h2 * 8:(h2 + 1) * 8, :],
                                src[:, :, k * 128:(k + 1) * 128])
                        for st in range(NTT):
                            tp = pA.tile([128, 128], bf16)
                            nc.tensor.transpose(tp, colblk[:, st, :], ident_b)
                            nc.vector.tensor_copy(xT[k][:, st * 128:(st + 1) * 128], tp)

                # ---- Phases B+C: per pair, QKV projection then attention ----
                with tc.tile_pool(name="qtp", bufs=2) as qtp, \
                     tc.tile_pool(name="ktp", bufs=2) as ktp, \
                     tc.tile_pool(name="vnp", bufs=2) as vnp, \
                     tc.tile_pool(name="stB", bufs=3) as sb_, \
                     tc.tile_pool(name="vt2", bufs=1) as vt2p, \
                     tc.tile_pool(name="expp", bufs=6) as expp, \
                     tc.tile_pool(name="rp", bufs=4) as rp, \
                     tc.tile_pool(name="ctxp", bufs=2) as ctxp, \
                     tc.tile_pool(name="psB", bufs=4, space="PSUM") as pB, \
                     tc.tile_pool(name="psCTX", bufs=1, space="PSUM") as psCTX:
                    for p in range(NP):
                        qt = qtp.tile([128, S], bf16, name="qt")
                        kt = ktp.tile([128, S], bf16, name="kt")
                        vn = vnp.tile([128, 2, NTT, 65], bf16, name="vn")
                        vt2 = vt2p.tile([128, S], bf16)
                        ctxn = ctxp.tile([128, S], bf16, name="ctxn")

                        # QKV projections (transposed, 2-head packed)
                        for wi, (bias_, dest) in enumerate(
                                ((bq_t[p], qt), (bk_t[p], kt), (None, vt2))):
                            wrs = []
                            for k in range(KT):
                                wf = sb_.tile([128, 128], bf16, name="wf", bufs=10)
                                nc.sync.dma_start(wf, wqkvg[p, wi, k])
                                wrs.append(wf)
                            for half in range(2):
                                pss = [pB.tile([128, 512], f32, name="pss", bufs=2)
                                       for _ in range(2)]
                                for k in range(KT):
                                    for i in range(2):
                                        nb = 2 * half + i
                                        nc.tensor.matmul(
                                            pss[i], wrs[k],
                                            xT[k][:, nb * 512:(nb + 1) * 512],
                                            start=(k == 0), stop=(k == KT - 1),
                                        )
                                for i in range(2):
                                    nb = 2 * half + i
                                    dslc = dest[:, nb * 512:(nb + 1) * 512]
                                    if bias_ is not None:
                                        nc.vector.tensor_scalar_add(dslc, pss[i], bias_)
                                    else:
                                        nc.vector.tensor_copy(dslc, pss[i])
                        # V back to natural [t, d] layout, split per head + ones col
                        for h in range(2):
                            nc.vector.tensor_copy(vn[:, h, :, 64:65], ones16)
                        for tt in range(NTT):
                            tp2 = pB.tile([128, 128], bf16, name="sc", bufs=4)
                            nc.tensor.transpose(tp2, vt2[:, tt * 128:(tt + 1) * 128], ident_b)
                            for h in range(2):
                                nc.vector.tensor_copy(
                                    vn[:, h, tt, 0:64], tp2[:, h * 64:(h + 1) * 64])

                        # attention for this pair
                        for qb in range(NQB):
                            T = 4 * (qb + 1)  # causal: t-tiles 0..T-1
                            cps = [psCTX.tile([65, 512], f32, name=f"cps{h}")
                                   for h in range(2)]
                            prev_exp = None
                            for tt in range(T):
                                scs = []
                                for h in range(2):
                                    sc = pB.tile([128, 512], f32, name="sc", bufs=4)
                                    nc.tensor.matmul(
                                        sc,
                                        kt[h * 64:(h + 1) * 64, tt * 128:(tt + 1) * 128],
                                        qt[h * 64:(h + 1) * 64, qb * 512:(qb + 1) * 512],
                                        start=True, stop=True,
                                    )
                                    scs.append(sc)
                                if prev_exp is not None:
                                    for h in range(2):
                                        nc.tensor.matmul(
                                            cps[h], vn[:, h, tt - 1, :], prev_exp[h],
                                            start=(tt - 1 == 0), stop=False,
                                        )
                                j = tt - 4 * qb  # >=0 on diagonal tiles
                                cur = []
                                for h in range(2):
                                    ex = expp.tile([128, 512], bf16)
                                    if j >= 1:
                                        nc.gpsimd.tensor_copy(
                                            ex[:, 0:j * 128], zeros_b[:, 0:j * 128])
                                    if j >= 0:
                                        nc.scalar.activation(
                                            ex[:, j * 128:512], scs[h][:, j * 128:512],
                                            Act.Exp, scale=0.125)
                                        nc.vector.tensor_mul(
                                            ex[:, j * 128:(j + 1) * 128],
                                            ex[:, j * 128:(j + 1) * 128], tri_b)
                                    else:
                                        nc.scalar.activation(ex, scs[h], Act.Exp, scale=0.125)
                                    cur.append(ex)
                                prev_exp = cur
                            for h in range(2):
                                nc.tensor.matmul(
                                    cps[h], vn[:, h, T - 1, :], prev_exp[h],
                                    start=(T - 1 == 0), stop=True,
                                )
                            # evict cps to SBUF fast (frees PSUM banks), then
                            # denominators (row 64) -> bcast -> reciprocal -> normalize
                            for h in range(2):
                                csb = rp.tile([65, 512], f32, name="csb", bufs=3)
                                nc.scalar.copy(csb, cps[h])
                                rh = rp.tile([1, 512], f32r, name="rh")
                                nc.vector.tensor_copy(rh, csb[64:65, :])
                                rb = pB.tile([64, 512], f32, name="sc", bufs=4)
                                nc.tensor.matmul(rb, ones_row, rh, start=True, stop=True)
                                rbs = rp.tile([64, 512], f32, name="rbs")
                                nc.vector.reciprocal(rbs, rb)
                                nc.vector.tensor_mul(
                                    ctxn[h * 64:(h + 1) * 64, qb * 512:(qb + 1) * 512],
                                    csb[0:64, :], rbs,
                                )
                        # share this pair's ctx with the peer core
                        nc.sync.dma_start(cin[p], ctxn)
                        allgather(PAIR_GROUPS, cin[p], cg[p])

                # ---- Phase D: output projection, my 512 columns, all 16 heads ----
                with tc.tile_pool(name="stD", bufs=3) as sd, \
                     tc.tile_pool(name="wo2", bufs=1) as wop, \
                     tc.tile_pool(name="ctxg", bufs=1) as cgp, \
                     tc.tile_pool(name="psD", bufs=4, space="PSUM") as pD:
                    wo_sb, ctx_sb = [], []
                    for gp in range(8):
                        g, p = gp // NP, gp % NP
                        w2 = wop.tile([128, 512], bf16, name=f"wo2_{gp}")
                        nc.sync.dma_start(w2, wog[gp])
                        wo_sb.append(w2)
                        c2 = cgp.tile([128, S], bf16, name=f"cg2_{gp}")
                        nc.sync.dma_start(c2, cg[p][g])
                        ctx_sb.append(c2)
                    for qt_i in range(NTT):
                        ob = sd.tile([128, 512], bf16, name="ob")
                        ps = pD.tile([128, 512], f32, name="psd")
                        for gp in range(8):
                            nc.tensor.matmul(
                                ps,
                                ctx_sb[gp][:, qt_i * 128:(qt_i + 1) * 128],
                                wo_sb[gp],
                                start=(gp == 0), stop=(gp == 7),
                            )
                        nc.vector.tensor_copy(ob, ps)
                        nc.sync.dma_start(OUT.ap()[qt_i * 128:(qt_i + 1) * 128, :], ob)

    nc.finalize()
    return nc


def _get_nc():
    global _NC
    if _NC is None:
        _NC = _build()
    return _NC


def _pack_w(Wh, bf):
    # [8, E, D] -> [NP, KT, 128, 128]; out[p,k,i,j] = Wh[2p + j//64, k*128+i, j%64]
    w = Wh.reshape(NP, 2, E, D)
    w = np.transpose(w, (0, 2, 1, 3)).reshape(NP, E, 128)
    w = w.reshape(NP, KT, 128, 128)
    return np.ascontiguousarray(w).astype(bf)


def build_inputs(x, Wq, bq, Wk, bk, Wv, bv, Wo, bo):
    """Per-core input dicts (list of 8) for the SPMD kernel."""
    import ml_dtypes
    bf = ml_dtypes.bfloat16

    x = np.asarray(x, dtype=np.float32)
    Wo = np.asarray(Wo, dtype=np.float32)
    xb = x.reshape(8 * 1024, E).astype(bf)

    # packed Wq/Wk/Wv per half: [NP, KT, 128, 128] each
    packs = []
    for hh in range(2):
        hsel = slice(hh * 8, hh * 8 + 8)
        packs.append([
            _pack_w(np.asarray(W, np.float32)[hsel], bf)
            for W in (Wq, Wk, Wv)])

    # Wo transposed, grouped by global head-pair: woT[gp, i, e] = Wo[e, 128*gp + i]
    woT = np.ascontiguousarray(Wo.T).astype(bf).reshape(8, 128, E)

    bq = np.asarray(bq, np.float32)
    bk = np.asarray(bk, np.float32)

    in_maps = []
    for c in range(8):
        b, hh = divmod(c, 2)
        pq, pk, pv = packs[hh]
        in_maps.append({
            "xs": xb[c * 1024:(c + 1) * 1024],
            "wqkvs": np.stack([pq[b], pk[b], pv[b]]),
            "wos": np.ascontiguousarray(
                woT[2 * b:2 * b + 2, :, hh * 512:(hh + 1) * 512]),
            "bqk": np.stack([
                bq[hh * 8:hh * 8 + 8].reshape(NP, 128, 1),
                bk[hh * 8:hh * 8 + 8].reshape(NP, 128, 1)]).astype(np.float32),
        })
    return in_maps


def assemble_output(parts, Wo, bo, bv):
    """parts: [8, S, 512] bf16 -> full [B, S, E] f32 with bias."""
    Wo = np.asarray(Wo, np.float32)
    bo_eff = (np.asarray(bo, np.float32)
              + np.asarray(bv, np.float32).reshape(-1) @ Wo.T)
    out = np.empty((B, S, E), np.float32)
    for c in range(8):
        b, hh = divmod(c, 2)
        out[b, :, hh * 512:(hh + 1) * 512] = parts[c]
    out += bo_eff[None, None, :]
    return out


def _make_runner(nc):
    """Cached jitted shard_map runner over 8 cores (axon / PJRT path).

    Output zero-buffers and the partition-id tensor are created on device
    inside the jitted function, so per call only the real inputs cross the
    host->device link.
    """
    import jax
    import jax.numpy as jnp
    from jax.sharding import Mesh, PartitionSpec, NamedSharding
    try:
        from jax.experimental.shard_map import shard_map
    except ImportError:
        from jax.shard_map import shard_map
    from concourse import bass2jax, mybir
    from concourse.bass2jax import _bass_exec_p, install_neuronx_cc_hook

    install_neuronx_cc_hook()

    partition_name = nc.partition_id_tensor.name if nc.partition_id_tensor else None
    dbg_name = nc.dbg_addr.name if nc.dbg_addr is not None else None

    in_names, out_names, out_avals = [], [], []
    for alloc in nc.m.functions[0].allocations:
        if not isinstance(alloc, mybir.MemoryLocationSet):
            continue
        name = alloc.memorylocations[0].name
        if alloc.kind == "ExternalInput":
            if name not in (partition_name, dbg_name):
                in_names.append(name)
        elif alloc.kind == "ExternalOutput":
            out_names.append(name)
            out_avals.append(jax.core.ShapedArray(
                tuple(alloc.tensor_shape), mybir.dt.np(alloc.dtype)))

    bind_names = list(in_names)
    if dbg_name is not None:
        bind_names.append(dbg_name)
    bind_names.extend(out_names)
    if partition_name is not None:
        bind_names.append(partition_name)

    def _body(*args):
        operands = list(args)
        if partition_name is not None:
            operands.append(bass2jax.partition_id_tensor())
        outs = _bass_exec_p.bind(
            *operands,
            out_avals=tuple(out_avals),
            in_names=tuple(bind_names),
            out_names=tuple(out_names),
            lowering_input_output_aliases=(),
            sim_require_finite=True,
            sim_require_nnan=True,
            nc=nc,
        )
        return tuple(outs)

    devices = jax.devices()[:8]
    mesh = Mesh(np.asarray(devices), ("core",))
    sharding = NamedSharding(mesh, PartitionSpec("core"))
    n_extra = (1 if dbg_name is not None else 0) + len(out_names)
    jitted = jax.jit(shard_map(
        _body, mesh=mesh,
        in_specs=(PartitionSpec("core"),) * (len(in_names) + n_extra),
        out_specs=(PartitionSpec("core"),) * len(out_names),
        check_rep=False,
    ))
    # device-resident zero buffers (output initializers + dbg), reused
    # across calls so they never cross the host->device link again
    extras = []
    if dbg_name is not None:
        extras.append(jax.device_put(np.zeros((8, 2), np.uint32), sharding))
    for a in out_avals:
        extras.append(jax.device_put(
            np.zeros((8 * a.shape[0], *a.shape[1:]), a.dtype), sharding))
    return jitted, in_names, out_names, out_avals, sharding, extras


def _get_runner():
    global _RUNNER
    if _RUNNER is None:
        _RUNNER = _make_runner(_get_nc())
    return _RUNNER


def _run_jax(in_maps):
    import jax
    jitted, in_names, out_names, out_avals, sharding, extras = _get_runner()
    gins = [
        jax.device_put(
            np.concatenate([np.asarray(im[nm]) for im in in_maps], axis=0),
            sharding)
        for nm in in_names]
    outs = jitted(*gins, *extras)
    res = []
    for i, nm in enumerate(out_names):
        arr = np.asarray(outs[i]).reshape(8, *out_avals[i].shape)
        res.append(arr)
    return dict(zip(out_names, res))


def kernel(x, Wq, bq, Wk, bk, Wv, bv, Wo, bo):
    import jax
    in_maps = build_inputs(x, Wq, bq, Wk, bk, Wv, bv, Wo, bo)

    use_jax = any(d.platform != "cpu" for d in jax.devices())
    if use_jax:
        outs = _run_jax(in_maps)
        parts = outs["out"]
    else:
        from concourse.bass_utils import run_bass_kernel_spmd
        res = run_bass_kernel_spmd(_get_nc(), in_maps, core_ids=list(range(8)))
        parts = np.stack([res.results[c]["out"] for c in range(8)])

    return assemble_output(parts, Wo, bo, bv).astype(np.float32)
